# revision 23
# baseline (speedup 1.0000x reference)
"""Multi-head causal attention (B=2, T=2048, E=1024, H=16, D=64) on 8 TRN2
NeuronCores, tensor-parallel over heads (2 heads per core).

v2 dataflow (bf16 SBUF datapath, fp32 PSUM accumulation):
  host:  xT = x^T [E, B*T] bf16 (same on all cores); wqkv_c [E, 384] bf16
         (this core's 2 heads of Wq|Wk|Wv); wproj_c = Wproj[128c:128c+128, :]
  1. qT[d,t], kT[d,t] = wqkv_c^T @ xT (PSUM accum over E tiles, 2 banks);
     v[s,d] computed DIRECTLY (lhsT = xt tile slice, rhs = Wv rows) into a
     third rotating bank - no PE transposes. Ones column per head appended in
     SBUF for the softmax denominator via the AV matmul.
  2. per (b, 512-t-block), heads sequential, per 128-s-tile:
       weiT[s,t] = kT^T q (K=64 bf16); Exp on ACT (scale=E^-0.5) PSUM->SBUF
       bf16; causal mask = multiplicative 0/1 tril on the diagonal 128-chunk
       (Pool) + zero-fill of below-diag chunks (Pool);
       avT_aug[65,t] += [v_h|1]^T @ expweiT (row 64 = denominator)
     denominator: 1/l = Exp(-Ln(l)) on ACT (Ln+Exp+Copy share one activation
     table -> no table swaps); K=2 PE matmul broadcasts 1/l across partitions;
     DVE multiplies av rows into the merged-head proj lhsT avT_sb [128,t] bf16.
  3. y_partial[t,e] = avT_sb^T @ wproj_c per 128-t-chunk; DVE copy psum->sbuf
     bf16; DMA out. host: y = sum_c y_partial_c + bproj.

Scheduling: emission interleaves QKV quanta and the PREVIOUS block's proj
matmuls into each attention block's si-steps so the tensor engine never goes
idle (keeps the PE p-state at 2.4 GHz and hides exp/normalize latency).
qkv block j: q/k pass emitted during attention block j-2, v pass during
block j-1; proj of block i-1 spread into block i.

Infra notes: this container's walrus accepts at most ONE semaphore wait
per instruction (_split_multi_waits hoists extras onto EventSemaphores),
and custom-DVE ops / gpsimd partition_broadcast do not compile ("ISA
wrong length"). The ACT engine runs only {Exp, Ln} - both live in the
natural_log_exp_and_others table so no ~1.3us table swaps occur. DVE
reciprocal is ~6.5ns/elem on HW - avoided entirely via Exp(-Ln(l)).
"""
import sys
import types

import numpy as np

B, T, E, H, D = 2, 2048, 1024, 16, 64
N_CORES = 8
HPC = H // N_CORES          # heads per core = 2
BT = B * T                  # 4096
DPC = HPC * D               # 128 head-dims per core
SCALE = 1.0 / float(np.sqrt(E))  # NOTE: reference scales by E**-0.5

NTB = T // 512              # 4 t-blocks per batch
NBLK = B * NTB              # 8 blocks total
NE = E // 128               # 8 e-tiles
NST = T // 128              # 16 s-tiles per batch


def _install_ntff_hook():
    if 'antenv.axon_hooks' in sys.modules:
        return
    try:
        sys.path.insert(0, '/root/.axon_site')
        from trn_agent_boot.trn_boot import _ntff_profile_via_ctypes
        hook = _ntff_profile_via_ctypes('/opt/axon/libaxon_pjrt.so')
        mod = types.ModuleType('antenv.axon_hooks')
        mod.get_axon_ntff_profile_hook = lambda: hook
        mod.set_axon_ntff_profile_hook = lambda h: None
        sys.modules['antenv.axon_hooks'] = mod
    except Exception:
        pass


def _split_multi_waits(nc, mybir):
    """This walrus build rejects >1 sync-wait per instruction. Hoist extra
    waits onto EventSemaphore instructions on the same engine just before."""
    for f in nc.m.functions:
        for bb in f.blocks:
            new_insts = []
            changed = False
            for inst in bb.instructions:
                si = inst.sync_info
                if si is not None and len(si.on_wait) > 1:
                    extra = list(si.on_wait[:-1])
                    keep = si.on_wait[-1]
                    for w in extra:
                        ev = mybir.InstEventSemaphore(
                            name=f"I-{nc.next_id()}", ins=[], outs=[])
                        ev.engine = inst.engine
                        ev.sync_info = mybir.SyncInfo(on_wait=[w], on_update=[])
                        new_insts.append(ev)
                    del si.on_wait[:]
                    si.on_wait.append(keep)
                    changed = True
                new_insts.append(inst)
            if changed:
                bb.instructions = new_insts


def _build_nc(split_waits=True):
    import concourse.bass as bass
    import concourse.mybir as mybir
    import concourse.tile as tile

    f32 = mybir.dt.float32
    bf16 = mybir.dt.bfloat16
    f8 = mybir.dt.float8e4
    u16 = mybir.dt.uint16
    EXP = mybir.ActivationFunctionType.Exp
    LN = mybir.ActivationFunctionType.Ln
    DR = mybir.MatmulPerfMode.DoubleRow
    ONE_BF16 = 16256            # bit pattern of bf16 1.0

    nc = bass.Bass('TRN2', num_devices=N_CORES)
    xt = nc.dram_tensor('xt', [E, BT], bf16, kind='ExternalInput')
    wqkv = nc.dram_tensor('wqkv', [E, 3 * DPC], bf16, kind='ExternalInput')
    wproj = nc.dram_tensor('wproj', [DPC, E], bf16, kind='ExternalInput')
    y = nc.dram_tensor('y', [BT, E], bf16, kind='ExternalOutput')
    # fp8 q/k staging for the partition->free pair repack (DoubleRow)
    qk8 = nc.dram_tensor('qk8', [NBLK, 2, 128, 512], f8, kind='Internal')

    with tile.TileContext(nc) as tc:
        with tc.tile_pool(name='consts', bufs=1) as consts, \
             tc.tile_pool(name='big', bufs=1) as big, \
             tc.tile_pool(name='work', bufs=1) as work, \
             tc.tile_pool(name='ps', bufs=1, space='PSUM') as ps:

            # ---- constants ----
            # multiplicative causal mask for the diagonal chunk of weiT [s,t]:
            # keep (1) where t >= s, 0 where t < s
            tmask_f = consts.tile([128, 128], f32)
            nc.gpsimd.memset(tmask_f[:], 1.0)
            nc.gpsimd.affine_select(
                out=tmask_f[:], in_=tmask_f[:],
                compare_op=mybir.AluOpType.is_ge,
                fill=0.0, base=0, pattern=[[1, 128]], channel_multiplier=-1)
            tmask = consts.tile([128, 128], bf16)
            nc.vector.tensor_copy(tmask[:], tmask_f[:])
            # sel2 [33,128]: row0 -> partitions 0-63 (head0), row32 -> 64-127
            # (partition offsets must be 32-aligned, hence rows 0/32)
            sel_f = consts.tile([33, 128], f32)
            nc.gpsimd.memset(sel_f[:], 0.0)
            nc.gpsimd.memset(sel_f[0:1, 0:64], 1.0)
            nc.gpsimd.memset(sel_f[32:33, 64:128], 1.0)
            sel2 = consts.tile([33, 128], bf16)
            nc.vector.tensor_copy(sel2[:], sel_f[:])

            # ---- weights ----
            wqkv_sb = [consts.tile([128, 3 * DPC], bf16, name=f'wqkv{k}')
                       for k in range(NE)]
            for k in range(NE):
                nc.sync.dma_start(out=wqkv_sb[k][:],
                                  in_=wqkv[k * 128:(k + 1) * 128, :])
            wproj_sb = consts.tile([DPC, E], bf16)
            nc.sync.dma_start(out=wproj_sb[:], in_=wproj[:])

            # ---- persistent activations ----
            # q/k live only as fp8e4 packed tiles for DoubleRow score matmuls:
            # [64, 2, 512]: partition 32h+p holds head h dims (2p, 2p+1) in
            # the pair free-dim (effective K=64 at 0.5 cycles/row).
            q8p = [big.tile([64, 2, 512], f8, name=f'q8p{j}')
                   for j in range(NBLK)]
            k8p = [big.tile([64, 2, 512], f8, name=f'k8p{j}')
                   for j in range(NBLK)]
            # v tiles [s,d] per 128-s-tile, layout [128, 2, 65]: per head 64
            # dims + ones column (softmax denominator via the AV matmul)
            v_sb = [big.tile([128, 2, 65], bf16, name=f'v{si}')
                    for si in range(2 * NST)]
            for si in range(2 * NST):
                nc.gpsimd.memset(v_sb[si][:, :, 64:65].bitcast(u16), ONE_BF16)

            # xt tiles for qkv block j live from its qk-pass until its v-pass
            xt_tiles = {}       # j -> list of 8 xt tiles

            # ---- qkv quanta ----
            def qk_quantum(j, k):
                """DMA one e-tile of xT and accumulate q/k projections."""
                def run(state):
                    if k == 0:
                        state['q_ps'] = ps.tile([128, 512], f32, tag='qkv',
                                                bufs=2, name=f'qps{j}')
                        state['k_ps'] = ps.tile([128, 512], f32, tag='qkv',
                                                bufs=2, name=f'kps{j}')
                        xt_tiles[j] = []
                    xt_t = work.tile([128, 512], bf16, tag='xt', bufs=18,
                                     name=f'xt{j}_{k}')
                    xt_tiles[j].append(xt_t)
                    ts = j * 512
                    nc.sync.dma_start(
                        out=xt_t[:], in_=xt[k * 128:(k + 1) * 128, ts:ts + 512])
                    st, sp = (k == 0), (k == NE - 1)
                    nc.tensor.matmul(state['q_ps'][:], wqkv_sb[k][:, 0:128],
                                     xt_t[:], start=st, stop=sp)
                    nc.tensor.matmul(state['k_ps'][:], wqkv_sb[k][:, 128:256],
                                     xt_t[:], start=st, stop=sp)
                    if sp:
                        # cast to fp8 and repack d-pairs into the free dim
                        # via a DRAM roundtrip (engines can't remap
                        # partitions; same-queue DMAs execute in order)
                        q8f = work.tile([128, 512], f8, tag='q8f', bufs=4,
                                        name=f'q8f{j}')
                        k8f = work.tile([128, 512], f8, tag='q8f', bufs=4,
                                        name=f'k8f{j}')
                        nc.vector.tensor_copy(q8f[:], state['q_ps'][:])
                        nc.vector.tensor_copy(k8f[:], state['k_ps'][:])
                        nc.sync.dma_start(out=qk8[j, 0], in_=q8f[:])
                        nc.sync.dma_start(out=qk8[j, 1], in_=k8f[:])
                        nc.sync.dma_start(
                            out=q8p[j][:],
                            in_=qk8[j, 0].rearrange('(h k i) t -> (h k) i t',
                                                    h=2, k=32, i=2))
                        nc.sync.dma_start(
                            out=k8p[j][:],
                            in_=qk8[j, 1].rearrange('(h k i) t -> (h k) i t',
                                                    h=2, k=32, i=2))
                return run

            def v_quantum(j, k):
                """Accumulate v[s,d] for one e-tile (4 column-slice groups)."""
                def run(state):
                    if k == 0:
                        state['v_ps'] = ps.tile([128, 512], f32, tag='qkv',
                                                bufs=2, name=f'vps{j}')
                    v_ps = state['v_ps']
                    xt_t = xt_tiles[j][k]
                    for sc in range(4):
                        # PSUM start=True lazily zeroes the WHOLE bank row,
                        # not just the addressed columns - so only the very
                        # first matmul into this bank may carry start=True.
                        nc.tensor.matmul(
                            v_ps[:, sc * 128:(sc + 1) * 128],
                            xt_t[:, sc * 128:(sc + 1) * 128],
                            wqkv_sb[k][:, 256:384],
                            start=(k == 0 and sc == 0), stop=(k == NE - 1),
                            skip_group_check=True)
                    if k == NE - 1:
                        for sc in range(4):
                            nc.vector.tensor_copy(
                                v_sb[j * 4 + sc][:, :, 0:64],
                                v_ps[:, sc * 128:(sc + 1) * 128].rearrange(
                                    'p (h e) -> p h e', h=2))
                return run

            # ---- attention steps for one (b, tb) block ----
            # software-pipelined: the AV matmul for chain element p is
            # emitted with the score matmul of element p+AVLAG, hiding the
            # exp (ACT) latency behind independent PE work.
            AVLAG = 2

            def attn_steps(b, tb):
                blk = b * NTB + tb
                n_si = 4 * (tb + 1)
                state = {'wt': {}}

                def start_block():
                    state['av'] = [
                        ps.tile([65, 512], f32, tag='av', bufs=3,
                                name=f'av{blk}_{h}') for h in range(HPC)]
                    # heads' ln(l) land on rows 0/32 (32-aligned partition
                    # offsets); zero-fill so Exp of untouched rows stays
                    # finite (sel2 zeros them out of the broadcast matmul)
                    lnl = work.tile([33, 512], f32, tag='lnl', bufs=2,
                                    name=f'lnl{blk}')
                    nc.gpsimd.memset(lnl[:], 0.0)
                    state['lnl'] = lnl

                def score_part(h, si):
                    sblk = b * NTB + si // 4
                    srem = (si % 4) * 128
                    woff = (si - 4 * tb) * 128 if si >= 4 * tb else 0
                    w_ps = ps.tile([128, 512], f32, tag='wei', bufs=3,
                                   name=f'wps{blk}_{h}_{si}')
                    nc.tensor.matmul(
                        w_ps[:],
                        k8p[sblk][32 * h:32 * h + 32, :, srem:srem + 128],
                        q8p[blk][32 * h:32 * h + 32, :, :],
                        start=True, stop=True, perf_mode=DR)
                    wt = work.tile([128, 512], bf16, tag='wt', bufs=24,
                                   name=f'wt{blk}_{h}_{si}')
                    if woff > 0:
                        nc.gpsimd.memset(
                            wt[:, 0:woff].bitcast(mybir.dt.uint32), 0)
                    nc.scalar.activation(wt[:, woff:512], w_ps[:, woff:512],
                                         EXP, scale=SCALE)
                    if si >= 4 * tb:
                        nc.gpsimd.tensor_mul(wt[:, woff:woff + 128],
                                             wt[:, woff:woff + 128],
                                             tmask[:])
                    state['wt'][(h, si)] = wt

                def av_part(h, si):
                    nc.tensor.matmul(
                        state['av'][h][:], v_sb[b * NST + si][:, h, :],
                        state['wt'].pop((h, si))[:],
                        start=(si == 0), stop=(si == n_si - 1))
                    if si == n_si - 1:
                        # softmax denominator row -> ln (ACT, no swap)
                        nc.scalar.activation(
                            state['lnl'][32 * h:32 * h + 1, :],
                            state['av'][h][64:65, :], LN)

                def finish_block():
                    # 1/l = exp(-ln l) on ACT; PE broadcast across partitions
                    rc_bf = work.tile([33, 512], bf16, tag='rc', bufs=2,
                                      name=f'rc{blk}')
                    nc.scalar.activation(rc_bf[:], state['lnl'][:], EXP,
                                         scale=-1.0)
                    bc_ps = ps.tile([128, 512], f32, tag='wei', bufs=3,
                                    name=f'bc{blk}')
                    nc.tensor.matmul(bc_ps[:], sel2[:], rc_bf[:],
                                     start=True, stop=True)
                    # DVE can't take two PSUM operands in one tensor_tensor;
                    # stage the broadcast through SBUF on ACT (Copy shares
                    # the Exp/Ln activation table - no swap)
                    bc_sb = work.tile([128, 512], f32, tag='bcs', bufs=2,
                                      name=f'bcs{blk}')
                    nc.scalar.copy(bc_sb[:], bc_ps[:])
                    avT_sb = work.tile([128, 512], bf16, tag='avT', bufs=3,
                                       name=f'avT{blk}')
                    for h in range(HPC):
                        hd = h * 64
                        nc.vector.tensor_mul(avT_sb[hd:hd + 64, :],
                                             state['av'][h][0:64, :],
                                             bc_sb[hd:hd + 64, :])
                    state['avT'] = avT_sb

                # heads sequential (not interleaved): the h1 chain starts
                # n_si steps in, guaranteeing the PREVIOUS block's deferred
                # normalize (which drains the psum bank h1 reuses) is
                # emitted first
                order = [(h, si) for h in range(HPC) for si in range(n_si)]

                def make_step(p):
                    def run():
                        score_part(*order[p])
                        if p >= AVLAG:
                            av_part(*order[p - AVLAG])
                    return run

                def make_tail(p):
                    return lambda: av_part(*order[p])

                steps = [start_block]
                steps += [make_step(p) for p in range(len(order))]
                steps += [make_tail(p) for p in range(len(order) - AVLAG,
                                                      len(order))]
                return steps, state, finish_block

            # ---- projection steps for one block (uses saved avT) ----
            def proj_steps(b, tb, state):
                t0 = b * T + tb * 512
                out = []
                for tc4 in range(4):
                    for eb in range(2):
                        def run(tc4=tc4, eb=eb):
                            y_ps = ps.tile([128, 512], f32, tag='wei', bufs=3,
                                           name=f'yps{b}_{tb}_{tc4}_{eb}')
                            nc.tensor.matmul(
                                y_ps[:],
                                state['avT'][:, tc4 * 128:(tc4 + 1) * 128],
                                wproj_sb[:, eb * 512:(eb + 1) * 512],
                                start=True, stop=True)
                            y_sb = work.tile([128, 512], bf16, tag='ysb',
                                             bufs=6, name=f'ysb{b}_{tb}_{tc4}_{eb}')
                            nc.vector.tensor_copy(y_sb[:], y_ps[:])
                            nc.sync.dma_start(
                                out=y[t0 + tc4 * 128:t0 + (tc4 + 1) * 128,
                                      eb * 512:(eb + 1) * 512],
                                in_=y_sb[:])
                        out.append(run)
                return out

            # ---- schedule ----
            qkv_state = {}      # j -> per-qkv-block state dict

            def qk_pass(j):
                qkv_state[j] = {}
                return [(lambda q=qk_quantum(j, k), j=j: q(qkv_state[j]))
                        for k in range(NE)]

            def v_pass(j):
                return [(lambda q=v_quantum(j, k), j=j: q(qkv_state[j]))
                        for k in range(NE)]

            # batch-interleaved block order keeps qkv filler work available
            # deep into the run (both batches' blocks are independent)
            block_order = [(b, tb) for tb in range(NTB) for b in range(B)]
            pos = {b * NTB + tb: i for i, (b, tb) in enumerate(block_order)}
            # qkv block j: qk-pass 2 positions early, v-pass 1 early
            qk_at = {}
            v_at = {}
            for j in range(NBLK):
                qk_at.setdefault(pos[j] - 2, []).append(j)
                v_at.setdefault(pos[j] - 1, []).append(j)

            # prologue: everything scheduled before position 0
            for p in sorted(k for k in qk_at if k < 0):
                for j in qk_at[p]:
                    for fn in qk_pass(j):
                        fn()
            for p in sorted(k for k in v_at if k < 0):
                for j in v_at[p]:
                    for fn in v_pass(j):
                        fn()

            prev_proj = []      # proj steps of previous attention block
            prev_finish = None  # deferred normalize of previous block
            for i, (b, tb) in enumerate(block_order):
                steps, state, finish = attn_steps(b, tb)
                quanta = []
                for j in v_at.get(i, []):
                    quanta += v_pass(j)
                for j in qk_at.get(i, []):
                    quanta += qk_pass(j)
                # previous block's normalize goes after a couple of quanta
                # (covers its ACT recip chain with PE work); its proj last
                fillers = quanta[:2]
                if prev_finish is not None:
                    fillers.append(prev_finish)
                fillers += quanta[2:]
                fillers += prev_proj
                nf, ns = len(fillers), len(steps)
                fi = 0
                for si_i, st in enumerate(steps):
                    st()
                    want = (si_i + 1) * nf // ns
                    while fi < want:
                        fillers[fi]()
                        fi += 1
                while fi < nf:
                    fillers[fi]()
                    fi += 1
                prev_finish = finish
                prev_proj = proj_steps(b, tb, state)
            prev_finish()
            for fn in prev_proj:
                fn()

    if split_waits:
        import concourse.mybir as mybir2
        _split_multi_waits(nc, mybir2)
    return nc


_CACHE = {}


def kernel(x, Wq, Wk, Wv, Wproj, bproj):
    _install_ntff_hook()
    import ml_dtypes
    from concourse.bass_utils import run_bass_kernel_spmd

    bf = ml_dtypes.bfloat16
    x = np.asarray(x, dtype=np.float32)
    Wq = np.asarray(Wq, dtype=np.float32)
    Wk = np.asarray(Wk, dtype=np.float32)
    Wv = np.asarray(Wv, dtype=np.float32)
    Wproj = np.asarray(Wproj, dtype=np.float32)
    bproj = np.asarray(bproj, dtype=np.float32)

    if 'nc' not in _CACHE:
        _CACHE['nc'] = _build_nc()
    nc = _CACHE['nc']

    xT = np.ascontiguousarray(x.reshape(BT, E).T).astype(bf)
    in_maps = []
    for c in range(N_CORES):
        h0 = HPC * c
        wqkv_c = np.concatenate(
            [Wq[h0], Wq[h0 + 1], Wk[h0], Wk[h0 + 1], Wv[h0], Wv[h0 + 1]],
            axis=1).astype(bf)                              # [E, 384]
        wproj_c = np.ascontiguousarray(Wproj[DPC * c: DPC * (c + 1)]).astype(bf)
        in_maps.append({'xt': xT, 'wqkv': np.ascontiguousarray(wqkv_c),
                        'wproj': wproj_c})

    res = run_bass_kernel_spmd(nc, in_maps, list(range(N_CORES)))
    ysum = np.zeros((BT, E), dtype=np.float64)
    for c in range(N_CORES):
        ysum += np.asarray(res.results[c]['y']).astype(np.float64)
    out = (ysum + bproj.astype(np.float64)).astype(np.float32)
    return out.reshape(B, T, E)


# revision 30
# speedup vs baseline: 1.1579x; 1.1579x over previous
"""Multi-head causal attention (B=2, T=2048, E=1024, H=16, D=64) on 8 TRN2
NeuronCores, tensor-parallel over heads (2 heads per core).

v2 dataflow (bf16 SBUF datapath, fp32 PSUM accumulation):
  host:  xT = x^T [E, B*T] bf16 (same on all cores); wqkv_c [E, 384] bf16
         (this core's 2 heads of Wq|Wk|Wv); wproj_c = Wproj[128c:128c+128, :]
  1. qT[d,t], kT[d,t] = wqkv_c^T @ xT (PSUM accum over E tiles, 2 banks);
     v[s,d] computed DIRECTLY (lhsT = xt tile slice, rhs = Wv rows) into a
     third rotating bank - no PE transposes. Ones column per head appended in
     SBUF for the softmax denominator via the AV matmul.
  2. per (b, 512-t-block), heads sequential, per 128-s-tile:
       weiT[s,t] = kT^T q (K=64 bf16); Exp on ACT (scale=E^-0.5) PSUM->SBUF
       bf16; causal mask = multiplicative 0/1 tril on the diagonal 128-chunk
       (Pool) + zero-fill of below-diag chunks (Pool);
       avT_aug[65,t] += [v_h|1]^T @ expweiT (row 64 = denominator)
     denominator: 1/l = Exp(-Ln(l)) on ACT (Ln+Exp+Copy share one activation
     table -> no table swaps); K=2 PE matmul broadcasts 1/l across partitions;
     DVE multiplies av rows into the merged-head proj lhsT avT_sb [128,t] bf16.
  3. y_partial[t,e] = avT_sb^T @ wproj_c per 128-t-chunk; DVE copy psum->sbuf
     bf16; DMA out. host: y = sum_c y_partial_c + bproj.

Scheduling: emission interleaves QKV quanta and the PREVIOUS block's proj
matmuls into each attention block's si-steps so the tensor engine never goes
idle (keeps the PE p-state at 2.4 GHz and hides exp/normalize latency).
qkv block j: q/k pass emitted during attention block j-2, v pass during
block j-1; proj of block i-1 spread into block i.

Infra notes: this container's walrus accepts at most ONE semaphore wait
per instruction (_split_multi_waits hoists extras onto EventSemaphores),
and custom-DVE ops / gpsimd partition_broadcast do not compile ("ISA
wrong length"). The ACT engine runs only {Exp, Ln} - both live in the
natural_log_exp_and_others table so no ~1.3us table swaps occur. DVE
reciprocal is ~6.5ns/elem on HW - avoided entirely via Exp(-Ln(l)).
"""
import sys
import types

import numpy as np

B, T, E, H, D = 2, 2048, 1024, 16, 64
N_CORES = 8
HPC = H // N_CORES          # heads per core = 2
BT = B * T                  # 4096
DPC = HPC * D               # 128 head-dims per core
SCALE = 1.0 / float(np.sqrt(E))  # NOTE: reference scales by E**-0.5

NTB = T // 512              # 4 t-blocks per batch
NBLK = B * NTB              # 8 blocks total
NE = E // 128               # 8 e-tiles
NST = T // 128              # 16 s-tiles per batch


def _install_ntff_hook():
    if 'antenv.axon_hooks' in sys.modules:
        return
    try:
        sys.path.insert(0, '/root/.axon_site')
        from trn_agent_boot.trn_boot import _ntff_profile_via_ctypes
        hook = _ntff_profile_via_ctypes('/opt/axon/libaxon_pjrt.so')
        mod = types.ModuleType('antenv.axon_hooks')
        mod.get_axon_ntff_profile_hook = lambda: hook
        mod.set_axon_ntff_profile_hook = lambda h: None
        sys.modules['antenv.axon_hooks'] = mod
    except Exception:
        pass


def _split_multi_waits(nc, mybir):
    """This walrus build rejects >1 sync-wait per instruction. Hoist extra
    waits onto EventSemaphore instructions on the same engine just before."""
    for f in nc.m.functions:
        for bb in f.blocks:
            new_insts = []
            changed = False
            for inst in bb.instructions:
                si = inst.sync_info
                if si is not None and len(si.on_wait) > 1:
                    extra = list(si.on_wait[:-1])
                    keep = si.on_wait[-1]
                    for w in extra:
                        ev = mybir.InstEventSemaphore(
                            name=f"I-{nc.next_id()}", ins=[], outs=[])
                        ev.engine = inst.engine
                        ev.sync_info = mybir.SyncInfo(on_wait=[w], on_update=[])
                        new_insts.append(ev)
                    del si.on_wait[:]
                    si.on_wait.append(keep)
                    changed = True
                new_insts.append(inst)
            if changed:
                bb.instructions = new_insts


def _build_nc(split_waits=True):
    import concourse.bass as bass
    import concourse.mybir as mybir
    import concourse.tile as tile

    f32 = mybir.dt.float32
    bf16 = mybir.dt.bfloat16
    f8 = mybir.dt.float8e4
    u16 = mybir.dt.uint16
    EXP = mybir.ActivationFunctionType.Exp
    LN = mybir.ActivationFunctionType.Ln
    DR = mybir.MatmulPerfMode.DoubleRow
    ONE_BF16 = 16256            # bit pattern of bf16 1.0

    nc = bass.Bass('TRN2', num_devices=N_CORES)
    xt = nc.dram_tensor('xt', [E, BT], bf16, kind='ExternalInput')
    wqkv = nc.dram_tensor('wqkv', [E, 3 * DPC], bf16, kind='ExternalInput')
    wproj = nc.dram_tensor('wproj', [DPC, E], bf16, kind='ExternalInput')
    y = nc.dram_tensor('y', [BT, E], bf16, kind='ExternalOutput')

    with tile.TileContext(nc) as tc:
        with tc.tile_pool(name='consts', bufs=1) as consts, \
             tc.tile_pool(name='big', bufs=1) as big, \
             tc.tile_pool(name='work', bufs=1) as work, \
             tc.tile_pool(name='ps', bufs=1, space='PSUM') as ps:

            # ---- constants ----
            # multiplicative causal mask for the diagonal chunk of weiT [s,t]:
            # keep (1) where t >= s, 0 where t < s
            tmask_f = consts.tile([128, 128], f32)
            nc.gpsimd.memset(tmask_f[:], 1.0)
            nc.gpsimd.affine_select(
                out=tmask_f[:], in_=tmask_f[:],
                compare_op=mybir.AluOpType.is_ge,
                fill=0.0, base=0, pattern=[[1, 128]], channel_multiplier=-1)
            tmask = consts.tile([128, 128], bf16)
            nc.vector.tensor_copy(tmask[:], tmask_f[:])
            # sel2 [33,128]: row0 -> partitions 0-63 (head0), row32 -> 64-127
            # (partition offsets must be 32-aligned, hence rows 0/32)
            sel_f = consts.tile([33, 128], f32)
            nc.gpsimd.memset(sel_f[:], 0.0)
            nc.gpsimd.memset(sel_f[0:1, 0:64], 1.0)
            nc.gpsimd.memset(sel_f[32:33, 64:128], 1.0)
            sel2 = consts.tile([33, 128], bf16)
            nc.vector.tensor_copy(sel2[:], sel_f[:])

            # ---- weights (DMAs emitted lazily inside the schedule so the
            # prologue's Sync queue stays short) ----
            wqkv_sb = [consts.tile([128, 3 * DPC], bf16, name=f'wqkv{k}')
                       for k in range(NE)]
            wproj_sb = consts.tile([DPC, E], bf16)

            # ---- persistent activations ----
            qT_sb = [big.tile([128, 512], bf16, name=f'q{j}')
                     for j in range(NBLK)]
            kT_sb = [big.tile([128, 512], bf16, name=f'k{j}')
                     for j in range(NBLK)]
            # v tiles [s,d] per 128-s-tile, layout [128, 2, 65]: per head 64
            # dims + ones column (softmax denominator via the AV matmul)
            v_sb = [big.tile([128, 2, 65], bf16, name=f'v{si}')
                    for si in range(2 * NST)]
            for si in range(2 * NST):
                nc.gpsimd.memset(v_sb[si][:, :, 64:65].bitcast(u16), ONE_BF16)

            # xt tiles for qkv block j live from its qk-pass until its v-pass
            xt_tiles = {}       # j -> list of 8 xt tiles
            wqkv_loaded = [False] * NE

            # ---- qkv quanta ----
            def qk_quantum(j, k):
                """DMA one e-tile of xT and accumulate q/k projections."""
                def run(state):
                    if k == 0:
                        state['q_ps'] = ps.tile([128, 512], f32, tag='qkv',
                                                bufs=2, name=f'qps{j}')
                        state['k_ps'] = ps.tile([128, 512], f32, tag='qkv',
                                                bufs=2, name=f'kps{j}')
                        xt_tiles[j] = []
                    if not wqkv_loaded[k]:
                        # lazy weight loads keep the prologue DMA queue short
                        wqkv_loaded[k] = True
                        nc.sync.dma_start(out=wqkv_sb[k][:],
                                          in_=wqkv[k * 128:(k + 1) * 128, :])
                    xt_t = work.tile([128, 512], bf16, tag='xt', bufs=18,
                                     name=f'xt{j}_{k}')
                    xt_tiles[j].append(xt_t)
                    ts = j * 512
                    nc.sync.dma_start(
                        out=xt_t[:], in_=xt[k * 128:(k + 1) * 128, ts:ts + 512])
                    st, sp = (k == 0), (k == NE - 1)
                    nc.tensor.matmul(state['q_ps'][:], wqkv_sb[k][:, 0:128],
                                     xt_t[:], start=st, stop=sp)
                    nc.tensor.matmul(state['k_ps'][:], wqkv_sb[k][:, 128:256],
                                     xt_t[:], start=st, stop=sp)
                    if sp:
                        nc.vector.tensor_copy(qT_sb[j][:], state['q_ps'][:])
                        nc.vector.tensor_copy(kT_sb[j][:], state['k_ps'][:])
                return run

            def v_quantum(j, k):
                """Accumulate v[s,d] for one e-tile (4 column-slice groups)."""
                def run(state):
                    if k == 0:
                        state['v_ps'] = ps.tile([128, 512], f32, tag='qkv',
                                                bufs=2, name=f'vps{j}')
                    v_ps = state['v_ps']
                    xt_t = xt_tiles[j][k]
                    for sc in range(4):
                        # PSUM start=True lazily zeroes the WHOLE bank row,
                        # not just the addressed columns - so only the very
                        # first matmul into this bank may carry start=True.
                        nc.tensor.matmul(
                            v_ps[:, sc * 128:(sc + 1) * 128],
                            xt_t[:, sc * 128:(sc + 1) * 128],
                            wqkv_sb[k][:, 256:384],
                            start=(k == 0 and sc == 0), stop=(k == NE - 1),
                            skip_group_check=True)
                    if k == NE - 1:
                        for sc in range(4):
                            nc.vector.tensor_copy(
                                v_sb[j * 4 + sc][:, :, 0:64],
                                v_ps[:, sc * 128:(sc + 1) * 128].rearrange(
                                    'p (h e) -> p h e', h=2))
                return run

            # ---- attention steps for one (b, tb) block ----
            # software-pipelined: the AV matmul for chain element p is
            # emitted with the score matmul of element p+AVLAG, hiding the
            # exp (ACT) latency behind independent PE work.
            AVLAG = 3

            def attn_steps(b, tb):
                blk = b * NTB + tb
                n_si = 4 * (tb + 1)
                state = {'wt': {}}

                def start_block():
                    state['av'] = [
                        ps.tile([65, 512], f32, tag='av', bufs=3,
                                name=f'av{blk}_{h}') for h in range(HPC)]
                    # heads' ln(l) land on rows 0/32 (32-aligned partition
                    # offsets); zero-fill so Exp of untouched rows stays
                    # finite (sel2 zeros them out of the broadcast matmul)
                    lnl = work.tile([33, 512], f32, tag='lnl', bufs=2,
                                    name=f'lnl{blk}')
                    nc.gpsimd.memset(lnl[:], 0.0)
                    state['lnl'] = lnl

                def score_part(h, si):
                    sblk = b * NTB + si // 4
                    srem = (si % 4) * 128
                    woff = (si - 4 * tb) * 128 if si >= 4 * tb else 0
                    hd = h * 64
                    w_ps = ps.tile([128, 512], f32, tag='wei', bufs=3,
                                   name=f'wps{blk}_{h}_{si}')
                    nc.tensor.matmul(
                        w_ps[:],
                        kT_sb[sblk][hd:hd + 64, srem:srem + 128],
                        qT_sb[blk][hd:hd + 64, :],
                        start=True, stop=True)
                    wt = work.tile([128, 512], bf16, tag='wt', bufs=24,
                                   name=f'wt{blk}_{h}_{si}')
                    if woff > 0:
                        nc.gpsimd.memset(
                            wt[:, 0:woff].bitcast(mybir.dt.uint32), 0)
                    nc.scalar.activation(wt[:, woff:512], w_ps[:, woff:512],
                                         EXP, scale=SCALE)
                    if si >= 4 * tb:
                        nc.gpsimd.tensor_mul(wt[:, woff:woff + 128],
                                             wt[:, woff:woff + 128],
                                             tmask[:])
                    state['wt'][(h, si)] = wt

                def av_part(h, si):
                    nc.tensor.matmul(
                        state['av'][h][:], v_sb[b * NST + si][:, h, :],
                        state['wt'].pop((h, si))[:],
                        start=(si == 0), stop=(si == n_si - 1))
                    if si == n_si - 1:
                        # softmax denominator row -> ln (ACT, no swap)
                        nc.scalar.activation(
                            state['lnl'][32 * h:32 * h + 1, :],
                            state['av'][h][64:65, :], LN)

                def finish_block():
                    # 1/l = exp(-ln l) on ACT; PE broadcast across partitions
                    rc_bf = work.tile([33, 512], bf16, tag='rc', bufs=2,
                                      name=f'rc{blk}')
                    nc.scalar.activation(rc_bf[:], state['lnl'][:], EXP,
                                         scale=-1.0)
                    bc_ps = ps.tile([128, 512], f32, tag='wei', bufs=3,
                                    name=f'bc{blk}')
                    nc.tensor.matmul(bc_ps[:], sel2[:], rc_bf[:],
                                     start=True, stop=True)
                    # DVE can't take two PSUM operands in one tensor_tensor;
                    # stage the broadcast through SBUF on ACT (Copy shares
                    # the Exp/Ln activation table - no swap)
                    bc_sb = work.tile([128, 512], f32, tag='bcs', bufs=2,
                                      name=f'bcs{blk}')
                    nc.scalar.copy(bc_sb[:], bc_ps[:])
                    avT_sb = work.tile([128, 512], bf16, tag='avT', bufs=3,
                                       name=f'avT{blk}')
                    for h in range(HPC):
                        hd = h * 64
                        nc.vector.tensor_mul(avT_sb[hd:hd + 64, :],
                                             state['av'][h][0:64, :],
                                             bc_sb[hd:hd + 64, :])
                    state['avT'] = avT_sb

                # heads sequential (not interleaved): the h1 chain starts
                # n_si steps in, guaranteeing the PREVIOUS block's deferred
                # normalize (which drains the psum bank h1 reuses) is
                # emitted first
                order = [(h, si) for h in range(HPC) for si in range(n_si)]

                def make_step(p):
                    def run():
                        score_part(*order[p])
                        if p >= AVLAG:
                            av_part(*order[p - AVLAG])
                    return run

                def make_tail(p):
                    return lambda: av_part(*order[p])

                steps = [start_block]
                steps += [make_step(p) for p in range(len(order))]
                steps += [make_tail(p) for p in range(len(order) - AVLAG,
                                                      len(order))]
                return steps, state, finish_block

            # ---- projection steps for one block (uses saved avT) ----
            def proj_steps(b, tb, state):
                t0 = b * T + tb * 512
                out = []
                for tc4 in range(4):
                    for eb in range(2):
                        def run(tc4=tc4, eb=eb):
                            y_ps = ps.tile([128, 512], f32, tag='wei', bufs=3,
                                           name=f'yps{b}_{tb}_{tc4}_{eb}')
                            nc.tensor.matmul(
                                y_ps[:],
                                state['avT'][:, tc4 * 128:(tc4 + 1) * 128],
                                wproj_sb[:, eb * 512:(eb + 1) * 512],
                                start=True, stop=True)
                            y_sb = work.tile([128, 512], bf16, tag='ysb',
                                             bufs=6, name=f'ysb{b}_{tb}_{tc4}_{eb}')
                            nc.vector.tensor_copy(y_sb[:], y_ps[:])
                            nc.sync.dma_start(
                                out=y[t0 + tc4 * 128:t0 + (tc4 + 1) * 128,
                                      eb * 512:(eb + 1) * 512],
                                in_=y_sb[:])
                        out.append(run)
                return out

            # ---- schedule ----
            qkv_state = {}      # j -> per-qkv-block state dict

            def qk_pass(j):
                qkv_state[j] = {}
                return [(lambda q=qk_quantum(j, k), j=j: q(qkv_state[j]))
                        for k in range(NE)]

            def v_pass(j):
                return [(lambda q=v_quantum(j, k), j=j: q(qkv_state[j]))
                        for k in range(NE)]

            # batch-interleaved block order keeps qkv filler work available
            # deep into the run (both batches' blocks are independent)
            block_order = [(b, tb) for tb in range(NTB) for b in range(B)]
            pos = {b * NTB + tb: i for i, (b, tb) in enumerate(block_order)}
            # qkv block j: qk-pass 2 positions early, v-pass 1 early
            qk_at = {}
            v_at = {}
            for j in range(NBLK):
                qk_at.setdefault(pos[j] - 2, []).append(j)
                v_at.setdefault(pos[j] - 1, []).append(j)

            # prologue: everything scheduled before position 0
            for p in sorted(k for k in qk_at if k < 0):
                for j in qk_at[p]:
                    for fn in qk_pass(j):
                        fn()
            for p in sorted(k for k in v_at if k < 0):
                for j in v_at[p]:
                    for fn in v_pass(j):
                        fn()
            # wproj is first needed by proj of block 0, emitted during
            # block 1 - load it after the prologue's critical DMAs
            nc.sync.dma_start(out=wproj_sb[:], in_=wproj[:])

            prev_proj = []      # proj steps of previous attention block
            prev_finish = None  # deferred normalize of previous block
            for i, (b, tb) in enumerate(block_order):
                steps, state, finish = attn_steps(b, tb)
                quanta = []
                for j in v_at.get(i, []):
                    quanta += v_pass(j)
                for j in qk_at.get(i, []):
                    quanta += qk_pass(j)
                # previous block's normalize goes after a couple of quanta
                # (covers its ACT recip chain with PE work); its proj last
                fillers = quanta[:2]
                if prev_finish is not None:
                    fillers.append(prev_finish)
                fillers += quanta[2:]
                fillers += prev_proj
                nf, ns = len(fillers), len(steps)
                fi = 0
                for si_i, st in enumerate(steps):
                    st()
                    want = (si_i + 1) * nf // ns
                    while fi < want:
                        fillers[fi]()
                        fi += 1
                while fi < nf:
                    fillers[fi]()
                    fi += 1
                prev_finish = finish
                prev_proj = proj_steps(b, tb, state)
            prev_finish()
            for fn in prev_proj:
                fn()

    if split_waits:
        import concourse.mybir as mybir2
        _split_multi_waits(nc, mybir2)
    return nc


_CACHE = {}


def kernel(x, Wq, Wk, Wv, Wproj, bproj):
    _install_ntff_hook()
    import ml_dtypes
    from concourse.bass_utils import run_bass_kernel_spmd

    bf = ml_dtypes.bfloat16
    x = np.asarray(x, dtype=np.float32)
    Wq = np.asarray(Wq, dtype=np.float32)
    Wk = np.asarray(Wk, dtype=np.float32)
    Wv = np.asarray(Wv, dtype=np.float32)
    Wproj = np.asarray(Wproj, dtype=np.float32)
    bproj = np.asarray(bproj, dtype=np.float32)

    if 'nc' not in _CACHE:
        _CACHE['nc'] = _build_nc()
    nc = _CACHE['nc']

    xT = np.ascontiguousarray(x.reshape(BT, E).T).astype(bf)
    in_maps = []
    for c in range(N_CORES):
        h0 = HPC * c
        wqkv_c = np.concatenate(
            [Wq[h0], Wq[h0 + 1], Wk[h0], Wk[h0 + 1], Wv[h0], Wv[h0 + 1]],
            axis=1).astype(bf)                              # [E, 384]
        wproj_c = np.ascontiguousarray(Wproj[DPC * c: DPC * (c + 1)]).astype(bf)
        in_maps.append({'xt': xT, 'wqkv': np.ascontiguousarray(wqkv_c),
                        'wproj': wproj_c})

    res = run_bass_kernel_spmd(nc, in_maps, list(range(N_CORES)))
    ysum = np.zeros((BT, E), dtype=np.float64)
    for c in range(N_CORES):
        ysum += np.asarray(res.results[c]['y']).astype(np.float64)
    out = (ysum + bproj.astype(np.float64)).astype(np.float32)
    return out.reshape(B, T, E)


# revision 37
# speedup vs baseline: 1.1617x; 1.0032x over previous
"""Multi-head causal attention (B=2, T=2048, E=1024, H=16, D=64) on 8 TRN2
NeuronCores, tensor-parallel over heads (2 heads per core).

v2 dataflow (bf16 SBUF datapath, fp32 PSUM accumulation):
  host:  xT = x^T [E, B*T] bf16 (same on all cores); wqkv_c [E, 384] bf16
         (this core's 2 heads of Wq|Wk|Wv); wproj_c = Wproj[128c:128c+128, :]
  1. qT[d,t], kT[d,t] = wqkv_c^T @ xT (PSUM accum over E tiles, 2 banks);
     v[s,d] computed DIRECTLY (lhsT = xt tile slice, rhs = Wv rows) into a
     third rotating bank - no PE transposes. Ones column per head appended in
     SBUF for the softmax denominator via the AV matmul.
  2. per (b, 512-t-block), heads sequential, per 128-s-tile:
       weiT[s,t] = kT^T q (K=64 bf16); Exp on ACT (scale=E^-0.5) PSUM->SBUF
       bf16; causal mask = multiplicative 0/1 tril on the diagonal 128-chunk
       (Pool) + zero-fill of below-diag chunks (Pool);
       avT_aug[65,t] += [v_h|1]^T @ expweiT (row 64 = denominator)
     denominator: 1/l = Exp(-Ln(l)) on ACT (Ln+Exp+Copy share one activation
     table -> no table swaps); K=2 PE matmul broadcasts 1/l across partitions;
     DVE multiplies av rows into the merged-head proj lhsT avT_sb [128,t] bf16.
  3. y_partial[t,e] = avT_sb^T @ wproj_c per 128-t-chunk; DVE copy psum->sbuf
     bf16; DMA out. host: y = sum_c y_partial_c + bproj.

Scheduling: emission interleaves QKV quanta and the PREVIOUS block's proj
matmuls into each attention block's si-steps so the tensor engine never goes
idle (keeps the PE p-state at 2.4 GHz and hides exp/normalize latency).
qkv block j: q/k pass emitted during attention block j-2, v pass during
block j-1; proj of block i-1 spread into block i.

Infra notes: this container's walrus accepts at most ONE semaphore wait
per instruction (_split_multi_waits hoists extras onto EventSemaphores),
and custom-DVE ops / gpsimd partition_broadcast do not compile ("ISA
wrong length"). The ACT engine runs only {Exp, Ln} - both live in the
natural_log_exp_and_others table so no ~1.3us table swaps occur. DVE
reciprocal is ~6.5ns/elem on HW - avoided entirely via Exp(-Ln(l)).
"""
import sys
import types

import numpy as np

B, T, E, H, D = 2, 2048, 1024, 16, 64
N_CORES = 8
HPC = H // N_CORES          # heads per core = 2
BT = B * T                  # 4096
DPC = HPC * D               # 128 head-dims per core
SCALE = 1.0 / float(np.sqrt(E))  # NOTE: reference scales by E**-0.5

NTB = T // 512              # 4 t-blocks per batch
NBLK = B * NTB              # 8 blocks total
NE = E // 128               # 8 e-tiles
NST = T // 128              # 16 s-tiles per batch


def _install_ntff_hook():
    if 'antenv.axon_hooks' in sys.modules:
        return
    try:
        sys.path.insert(0, '/root/.axon_site')
        from trn_agent_boot.trn_boot import _ntff_profile_via_ctypes
        hook = _ntff_profile_via_ctypes('/opt/axon/libaxon_pjrt.so')
        mod = types.ModuleType('antenv.axon_hooks')
        mod.get_axon_ntff_profile_hook = lambda: hook
        mod.set_axon_ntff_profile_hook = lambda h: None
        sys.modules['antenv.axon_hooks'] = mod
    except Exception:
        pass


def _split_multi_waits(nc, mybir):
    """This walrus build rejects >1 sync-wait per instruction. Hoist extra
    waits onto EventSemaphore instructions on the same engine just before."""
    for f in nc.m.functions:
        for bb in f.blocks:
            new_insts = []
            changed = False
            for inst in bb.instructions:
                si = inst.sync_info
                if si is not None and len(si.on_wait) > 1:
                    extra = list(si.on_wait[:-1])
                    keep = si.on_wait[-1]
                    for w in extra:
                        ev = mybir.InstEventSemaphore(
                            name=f"I-{nc.next_id()}", ins=[], outs=[])
                        ev.engine = inst.engine
                        ev.sync_info = mybir.SyncInfo(on_wait=[w], on_update=[])
                        new_insts.append(ev)
                    del si.on_wait[:]
                    si.on_wait.append(keep)
                    changed = True
                new_insts.append(inst)
            if changed:
                bb.instructions = new_insts


def _build_nc(split_waits=True):
    import concourse.bass as bass
    import concourse.mybir as mybir
    import concourse.tile as tile

    f32 = mybir.dt.float32
    bf16 = mybir.dt.bfloat16
    f8 = mybir.dt.float8e4
    u16 = mybir.dt.uint16
    EXP = mybir.ActivationFunctionType.Exp
    LN = mybir.ActivationFunctionType.Ln
    DR = mybir.MatmulPerfMode.DoubleRow
    ONE_BF16 = 16256            # bit pattern of bf16 1.0

    nc = bass.Bass('TRN2', num_devices=N_CORES)
    xt = nc.dram_tensor('xt', [E, BT], bf16, kind='ExternalInput')
    wqkv = nc.dram_tensor('wqkv', [E, 3 * DPC], bf16, kind='ExternalInput')
    wproj = nc.dram_tensor('wproj', [DPC, E], bf16, kind='ExternalInput')
    y = nc.dram_tensor('y', [BT, E], bf16, kind='ExternalOutput')

    with tile.TileContext(nc) as tc:
        with tc.tile_pool(name='consts', bufs=1) as consts, \
             tc.tile_pool(name='big', bufs=1) as big, \
             tc.tile_pool(name='work', bufs=1) as work, \
             tc.tile_pool(name='ps', bufs=1, space='PSUM') as ps:

            # ---- constants ----
            # multiplicative causal mask for the diagonal chunk of weiT [s,t]:
            # keep (1) where t >= s, 0 where t < s
            tmask_f = consts.tile([128, 128], f32)
            nc.gpsimd.memset(tmask_f[:], 1.0)
            nc.gpsimd.affine_select(
                out=tmask_f[:], in_=tmask_f[:],
                compare_op=mybir.AluOpType.is_ge,
                fill=0.0, base=0, pattern=[[1, 128]], channel_multiplier=-1)
            tmask = consts.tile([128, 128], bf16)
            nc.vector.tensor_copy(tmask[:], tmask_f[:])
            # sel2 [33,128]: row0 -> partitions 0-63 (head0), row32 -> 64-127
            # (partition offsets must be 32-aligned, hence rows 0/32)
            sel_f = consts.tile([33, 128], f32)
            nc.gpsimd.memset(sel_f[:], 0.0)
            nc.gpsimd.memset(sel_f[0:1, 0:64], 1.0)
            nc.gpsimd.memset(sel_f[32:33, 64:128], 1.0)
            sel2 = consts.tile([33, 128], bf16)
            nc.vector.tensor_copy(sel2[:], sel_f[:])

            # ---- weights (DMAs emitted lazily inside the schedule so the
            # prologue's Sync queue stays short) ----
            wqkv_sb = [consts.tile([128, 3 * DPC], bf16, name=f'wqkv{k}')
                       for k in range(NE)]
            wproj_sb = consts.tile([DPC, E], bf16)

            # ---- persistent activations ----
            qT_sb = [big.tile([128, 512], bf16, name=f'q{j}')
                     for j in range(NBLK)]
            kT_sb = [big.tile([128, 512], bf16, name=f'k{j}')
                     for j in range(NBLK)]
            # v tiles [s,d] per 128-s-tile, layout [128, 2, 65]: per head 64
            # dims + ones column (softmax denominator via the AV matmul)
            v_sb = [big.tile([128, 2, 65], bf16, name=f'v{si}')
                    for si in range(2 * NST)]
            for si in range(2 * NST):
                nc.gpsimd.memset(v_sb[si][:, :, 64:65].bitcast(u16), ONE_BF16)

            # xt tiles for qkv block j live from its qk-pass until its v-pass
            xt_tiles = {}       # j -> list of 8 xt tiles
            wqkv_loaded = [False] * NE

            # ---- qkv quanta ----
            def qk_quantum(j, k):
                """DMA one e-tile of xT and accumulate q/k projections."""
                def run(state):
                    if k == 0:
                        state['q_ps'] = ps.tile([128, 512], f32, tag='qkv',
                                                bufs=2, name=f'qps{j}')
                        state['k_ps'] = ps.tile([128, 512], f32, tag='qkv',
                                                bufs=2, name=f'kps{j}')
                        xt_tiles[j] = []
                    if not wqkv_loaded[k]:
                        # lazy weight loads keep the prologue DMA queue short
                        wqkv_loaded[k] = True
                        nc.sync.dma_start(out=wqkv_sb[k][:],
                                          in_=wqkv[k * 128:(k + 1) * 128, :])
                    xt_t = work.tile([128, 512], bf16, tag='xt', bufs=18,
                                     name=f'xt{j}_{k}')
                    xt_tiles[j].append(xt_t)
                    ts = j * 512
                    nc.sync.dma_start(
                        out=xt_t[:], in_=xt[k * 128:(k + 1) * 128, ts:ts + 512])
                    st, sp = (k == 0), (k == NE - 1)
                    nc.tensor.matmul(state['q_ps'][:], wqkv_sb[k][:, 0:128],
                                     xt_t[:], start=st, stop=sp)
                    nc.tensor.matmul(state['k_ps'][:], wqkv_sb[k][:, 128:256],
                                     xt_t[:], start=st, stop=sp)
                    if sp:
                        nc.vector.tensor_copy(qT_sb[j][:], state['q_ps'][:])
                        nc.vector.tensor_copy(kT_sb[j][:], state['k_ps'][:])
                return run

            def v_quantum(j, k):
                """Accumulate v[s,d] for one e-tile (4 column-slice groups)."""
                def run(state):
                    if k == 0:
                        state['v_ps'] = ps.tile([128, 512], f32, tag='qkv',
                                                bufs=2, name=f'vps{j}')
                    v_ps = state['v_ps']
                    xt_t = xt_tiles[j][k]
                    for sc in range(4):
                        # PSUM start=True lazily zeroes the WHOLE bank row,
                        # not just the addressed columns - so only the very
                        # first matmul into this bank may carry start=True.
                        nc.tensor.matmul(
                            v_ps[:, sc * 128:(sc + 1) * 128],
                            xt_t[:, sc * 128:(sc + 1) * 128],
                            wqkv_sb[k][:, 256:384],
                            start=(k == 0 and sc == 0), stop=(k == NE - 1),
                            skip_group_check=True)
                    if k == NE - 1:
                        for sc in range(4):
                            nc.vector.tensor_copy(
                                v_sb[j * 4 + sc][:, :, 0:64],
                                v_ps[:, sc * 128:(sc + 1) * 128].rearrange(
                                    'p (h e) -> p h e', h=2))
                return run

            # ---- attention steps for one (b, tb) block ----
            # software-pipelined: the AV matmul for chain element p is
            # emitted with the score matmul of element p+AVLAG, hiding the
            # exp (ACT) latency behind independent PE work. PAIRED batches
            # same-shape matmuls (score,score then av,av) to cut PE config
            # switches.
            AVLAG = 3
            PAIRED = True

            def attn_steps(b, tb, split_tail=False):
                blk = b * NTB + tb
                n_si = 4 * (tb + 1)
                state = {'wt': {}}

                def start_block():
                    state['av'] = [
                        ps.tile([65, 512], f32, tag='av', bufs=3,
                                name=f'av{blk}_{h}') for h in range(HPC)]
                    # heads' ln(l) land on rows 0/32 (32-aligned partition
                    # offsets); zero-fill so Exp of untouched rows stays
                    # finite (sel2 zeros them out of the broadcast matmul)
                    lnl = work.tile([33, 512], f32, tag='lnl', bufs=2,
                                    name=f'lnl{blk}')
                    nc.gpsimd.memset(lnl[:], 0.0)
                    state['lnl'] = lnl

                def score_part(h, si):
                    sblk = b * NTB + si // 4
                    srem = (si % 4) * 128
                    woff = (si - 4 * tb) * 128 if si >= 4 * tb else 0
                    hd = h * 64
                    w_ps = ps.tile([128, 512], f32, tag='wei', bufs=3,
                                   name=f'wps{blk}_{h}_{si}')
                    nc.tensor.matmul(
                        w_ps[:],
                        kT_sb[sblk][hd:hd + 64, srem:srem + 128],
                        qT_sb[blk][hd:hd + 64, :],
                        start=True, stop=True)
                    wt = work.tile([128, 512], bf16, tag='wt', bufs=24,
                                   name=f'wt{blk}_{h}_{si}')
                    if woff > 0:
                        nc.gpsimd.memset(
                            wt[:, 0:woff].bitcast(mybir.dt.uint32), 0)
                    nc.scalar.activation(wt[:, woff:512], w_ps[:, woff:512],
                                         EXP, scale=SCALE)
                    if si >= 4 * tb:
                        nc.gpsimd.tensor_mul(wt[:, woff:woff + 128],
                                             wt[:, woff:woff + 128],
                                             tmask[:])
                    state['wt'][(h, si)] = wt

                def av_part(h, si):
                    nc.tensor.matmul(
                        state['av'][h][:], v_sb[b * NST + si][:, h, :],
                        state['wt'].pop((h, si))[:],
                        start=(si == 0), stop=(si == n_si - 1))
                    if si == n_si - 1:
                        # softmax denominator row -> ln (ACT, no swap)
                        nc.scalar.activation(
                            state['lnl'][32 * h:32 * h + 1, :],
                            state['av'][h][64:65, :], LN)

                def finish_block():
                    # 1/l = exp(-ln l) on ACT; PE broadcast across partitions
                    rc_bf = work.tile([33, 512], bf16, tag='rc', bufs=2,
                                      name=f'rc{blk}')
                    nc.scalar.activation(rc_bf[:], state['lnl'][:], EXP,
                                         scale=-1.0)
                    bc_ps = ps.tile([128, 512], f32, tag='wei', bufs=3,
                                    name=f'bc{blk}')
                    nc.tensor.matmul(bc_ps[:], sel2[:], rc_bf[:],
                                     start=True, stop=True)
                    # DVE can't take two PSUM operands in one tensor_tensor;
                    # stage the broadcast through SBUF on ACT (Copy shares
                    # the Exp/Ln activation table - no swap)
                    bc_sb = work.tile([128, 512], f32, tag='bcs', bufs=2,
                                      name=f'bcs{blk}')
                    nc.scalar.copy(bc_sb[:], bc_ps[:])
                    avT_sb = work.tile([128, 512], bf16, tag='avT', bufs=3,
                                       name=f'avT{blk}')
                    for h in range(HPC):
                        hd = h * 64
                        nc.vector.tensor_mul(avT_sb[hd:hd + 64, :],
                                             state['av'][h][0:64, :],
                                             bc_sb[hd:hd + 64, :])
                    state['avT'] = avT_sb

                # heads sequential (not interleaved): the h1 chain starts
                # n_si steps in, guaranteeing the PREVIOUS block's deferred
                # normalize (which drains the psum bank h1 reuses) is
                # emitted first
                def finish_head(h):
                    """Per-head normalize chain (final block: lets the h0
                    half of the projection start while h1 still computes)."""
                    def run():
                        hd = h * 64
                        rc_h = work.tile([1, 512], bf16, tag='rch', bufs=2,
                                         name=f'rch{blk}_{h}')
                        nc.scalar.activation(
                            rc_h[:], state['lnl'][32 * h:32 * h + 1, :],
                            EXP, scale=-1.0)
                        bch_ps = ps.tile([128, 512], f32, tag='wei', bufs=3,
                                         name=f'bch{blk}_{h}')
                        nc.tensor.matmul(bch_ps[hd:hd + 64, :],
                                         sel2[0:1, 0:64], rc_h[:],
                                         start=True, stop=True)
                        bch_sb = work.tile([128, 512], f32, tag='bcs', bufs=2,
                                           name=f'bchs{blk}_{h}')
                        nc.scalar.copy(bch_sb[hd:hd + 64, :],
                                       bch_ps[hd:hd + 64, :])
                        nc.vector.tensor_mul(state['avTs'][hd:hd + 64, :],
                                             state['av'][h][0:64, :],
                                             bch_sb[hd:hd + 64, :])
                    return run

                def proj_half(h, y0_tiles):
                    t0 = b * T + tb * 512
                    out = []
                    for tc4 in range(4):
                        for eb in range(2):
                            def run(tc4=tc4, eb=eb):
                                hd = h * 64
                                y_ps = ps.tile([128, 512], f32, tag='wei',
                                               bufs=3,
                                               name=f'yph{blk}_{h}_{tc4}_{eb}')
                                nc.tensor.matmul(
                                    y_ps[:],
                                    state['avTs'][hd:hd + 64,
                                                  tc4 * 128:(tc4 + 1) * 128],
                                    wproj_sb[hd:hd + 64,
                                             eb * 512:(eb + 1) * 512],
                                    start=True, stop=True)
                                if h == 0:
                                    y0 = work.tile([128, 512], bf16, tag='y0',
                                                   bufs=8,
                                                   name=f'y0_{blk}_{tc4}_{eb}')
                                    nc.vector.tensor_copy(y0[:], y_ps[:])
                                    y0_tiles[(tc4, eb)] = y0
                                else:
                                    y_sb = work.tile([128, 512], bf16,
                                                     tag='ysb', bufs=6,
                                                     name=f'ysh{blk}_{tc4}_{eb}')
                                    nc.vector.tensor_add(
                                        y_sb[:], y_ps[:],
                                        y0_tiles[(tc4, eb)][:])
                                    nc.sync.dma_start(
                                        out=y[t0 + tc4 * 128:
                                              t0 + (tc4 + 1) * 128,
                                              eb * 512:(eb + 1) * 512],
                                        in_=y_sb[:])
                            out.append(run)
                    return out

                def paired_pipeline(sub):
                    """Score/av pipeline over one head's elements."""
                    out = []

                    def make(pk):
                        def run():
                            for p in (2 * pk, 2 * pk + 1):
                                if p < len(sub):
                                    score_part(*sub[p])
                            for p in (2 * pk - AVLAG - 1, 2 * pk - AVLAG):
                                if 0 <= p:
                                    av_part(*sub[p])
                        return run
                    npk = (len(sub) + 1) // 2
                    out += [make(pk) for pk in range(npk)]

                    def tail():
                        for p in range(2 * npk - AVLAG - 1, len(sub)):
                            if 0 <= p:
                                av_part(*sub[p])
                    out.append(tail)
                    return out

                if split_tail:
                    def start_split():
                        start_block()
                        state['avTs'] = work.tile([128, 512], bf16, tag='avT',
                                                  bufs=3, name=f'avTs{blk}')
                    y0_tiles = {}
                    steps = [start_split]
                    steps += paired_pipeline([(0, si) for si in range(n_si)])
                    steps += [finish_head(0)]
                    h1_steps = paired_pipeline([(1, si) for si in range(n_si)])
                    p0 = proj_half(0, y0_tiles)
                    # weave h0's proj halves into h1's si-steps
                    merged = []
                    k = 0
                    for si_i, st in enumerate(h1_steps):
                        merged.append(st)
                        want = (si_i + 1) * len(p0) // len(h1_steps)
                        while k < want:
                            merged.append(p0[k])
                            k += 1
                    merged += p0[k:]
                    steps += merged
                    steps += [finish_head(1)]
                    return steps, state, None, proj_half(1, y0_tiles)

                order = [(h, si) for h in range(HPC) for si in range(n_si)]
                if PAIRED:
                    # emit same-shape matmuls in pairs: score,score,av,av
                    def make_step(pk):
                        def run():
                            for p in (2 * pk, 2 * pk + 1):
                                if p < len(order):
                                    score_part(*order[p])
                            for p in (2 * pk - AVLAG - 1, 2 * pk - AVLAG):
                                if 0 <= p:
                                    av_part(*order[p])
                        return run

                    npk = (len(order) + 1) // 2
                    steps = [start_block]
                    steps += [make_step(pk) for pk in range(npk)]

                    def tail():
                        for p in range(2 * npk - AVLAG - 1, len(order)):
                            if 0 <= p:
                                av_part(*order[p])
                    steps.append(tail)
                else:
                    def make_step(p):
                        def run():
                            score_part(*order[p])
                            if p >= AVLAG:
                                av_part(*order[p - AVLAG])
                        return run

                    def make_tail(p):
                        return lambda: av_part(*order[p])

                    steps = [start_block]
                    steps += [make_step(p) for p in range(len(order))]
                    steps += [make_tail(p) for p in range(len(order) - AVLAG,
                                                          len(order))]
                return steps, state, finish_block, None

            # ---- projection steps for one block (uses saved avT) ----
            def proj_steps(b, tb, state):
                t0 = b * T + tb * 512
                out = []
                for tc4 in range(4):
                    for eb in range(2):
                        def run(tc4=tc4, eb=eb):
                            y_ps = ps.tile([128, 512], f32, tag='wei', bufs=3,
                                           name=f'yps{b}_{tb}_{tc4}_{eb}')
                            nc.tensor.matmul(
                                y_ps[:],
                                state['avT'][:, tc4 * 128:(tc4 + 1) * 128],
                                wproj_sb[:, eb * 512:(eb + 1) * 512],
                                start=True, stop=True)
                            y_sb = work.tile([128, 512], bf16, tag='ysb',
                                             bufs=6, name=f'ysb{b}_{tb}_{tc4}_{eb}')
                            nc.vector.tensor_copy(y_sb[:], y_ps[:])
                            nc.sync.dma_start(
                                out=y[t0 + tc4 * 128:t0 + (tc4 + 1) * 128,
                                      eb * 512:(eb + 1) * 512],
                                in_=y_sb[:])
                        out.append(run)
                return out

            # ---- schedule ----
            qkv_state = {}      # j -> per-qkv-block state dict

            def qk_pass(j):
                qkv_state[j] = {}
                return [(lambda q=qk_quantum(j, k), j=j: q(qkv_state[j]))
                        for k in range(NE)]

            def v_pass(j):
                return [(lambda q=v_quantum(j, k), j=j: q(qkv_state[j]))
                        for k in range(NE)]

            # batch-interleaved block order keeps qkv filler work available
            # deep into the run (both batches' blocks are independent)
            block_order = [(b, tb) for tb in range(NTB) for b in range(B)]
            pos = {b * NTB + tb: i for i, (b, tb) in enumerate(block_order)}
            # qkv block j: qk-pass 2 positions early, v-pass 1 early
            qk_at = {}
            v_at = {}
            for j in range(NBLK):
                qk_at.setdefault(pos[j] - 2, []).append(j)
                v_at.setdefault(pos[j] - 1, []).append(j)

            # prologue: everything scheduled before position 0
            for p in sorted(k for k in qk_at if k < 0):
                for j in qk_at[p]:
                    for fn in qk_pass(j):
                        fn()
            for p in sorted(k for k in v_at if k < 0):
                for j in v_at[p]:
                    for fn in v_pass(j):
                        fn()
            # wproj is first needed by proj of block 0, emitted during
            # block 1 - load it after the prologue's critical DMAs
            nc.sync.dma_start(out=wproj_sb[:], in_=wproj[:])

            prev_proj = []      # proj steps of previous attention block
            prev_finish = None  # deferred normalize of previous block
            for i, (b, tb) in enumerate(block_order):
                steps, state, finish, final_proj = attn_steps(
                    b, tb, split_tail=(i == NBLK - 1))
                quanta = []
                for j in v_at.get(i, []):
                    quanta += v_pass(j)
                for j in qk_at.get(i, []):
                    quanta += qk_pass(j)
                # previous block's normalize goes after a couple of quanta
                # (covers its ACT recip chain with PE work); its proj last
                fillers = quanta[:2]
                if prev_finish is not None:
                    fillers.append(prev_finish)
                fillers += quanta[2:]
                fillers += prev_proj
                nf, ns = len(fillers), len(steps)
                fi = 0
                for si_i, st in enumerate(steps):
                    st()
                    want = (si_i + 1) * nf // ns
                    while fi < want:
                        fillers[fi]()
                        fi += 1
                while fi < nf:
                    fillers[fi]()
                    fi += 1
                prev_finish = finish
                prev_proj = (final_proj if final_proj is not None
                             else proj_steps(b, tb, state))
            if prev_finish is not None:
                prev_finish()
            for fn in prev_proj:
                fn()

    if split_waits:
        import concourse.mybir as mybir2
        _split_multi_waits(nc, mybir2)
    return nc


_CACHE = {}


def kernel(x, Wq, Wk, Wv, Wproj, bproj):
    _install_ntff_hook()
    import ml_dtypes
    from concourse.bass_utils import run_bass_kernel_spmd

    bf = ml_dtypes.bfloat16
    x = np.asarray(x, dtype=np.float32)
    Wq = np.asarray(Wq, dtype=np.float32)
    Wk = np.asarray(Wk, dtype=np.float32)
    Wv = np.asarray(Wv, dtype=np.float32)
    Wproj = np.asarray(Wproj, dtype=np.float32)
    bproj = np.asarray(bproj, dtype=np.float32)

    if 'nc' not in _CACHE:
        _CACHE['nc'] = _build_nc()
    nc = _CACHE['nc']

    xT = np.ascontiguousarray(x.reshape(BT, E).T).astype(bf)
    in_maps = []
    for c in range(N_CORES):
        h0 = HPC * c
        wqkv_c = np.concatenate(
            [Wq[h0], Wq[h0 + 1], Wk[h0], Wk[h0 + 1], Wv[h0], Wv[h0 + 1]],
            axis=1).astype(bf)                              # [E, 384]
        wproj_c = np.ascontiguousarray(Wproj[DPC * c: DPC * (c + 1)]).astype(bf)
        in_maps.append({'xt': xT, 'wqkv': np.ascontiguousarray(wqkv_c),
                        'wproj': wproj_c})

    res = run_bass_kernel_spmd(nc, in_maps, list(range(N_CORES)))
    ysum = np.zeros((BT, E), dtype=np.float64)
    for c in range(N_CORES):
        ysum += np.asarray(res.results[c]['y']).astype(np.float64)
    out = (ysum + bproj.astype(np.float64)).astype(np.float32)
    return out.reshape(B, T, E)
